# revision 52
# baseline (speedup 1.0000x reference)
"""Trainium2 Bass kernel for 2-layer GAT + graph pooling + MLP.

Sharding: nodes dst-sharded across 8 cores (6250 contiguous nodes each).
Each core replicates the node transform into a per-core-ROTATED fat table in
HBM (row r = node (core_base + r) % N). Table row = 512B: [h 256xfp8e4 |
esrc 4xbf16 | edst 4xbf16 | pad]; the per-edge src gather moves only the
first 264B. Two int16-safe half tables (rows [0,32768) and the rest), with
the B half written first so B-half gathers overlap the rest of phase A.

Edge phase processes windows of 128 dst nodes in PAIRS. One-hot matrices
(edge->dst selector oh, and its transpose ohT for per-edge dst-logit
reconstruction) are pure graph topology: prebuilt on host in fp8 and DMA
loaded, not computed on device. Per-edge chain: ex = exp(lrelu(esrc+edst));
msg = h_fp8 * ex (DVE, fp8 in); segment-sum + denominator via one
oh-selector matmul per tile into PSUM; normalize + bias + relu on DVE.

3 launches: L1, L2 (+ pooling partials), L3 (MLP head, single packed DMA).
Host work is layout/topology only: transposes, rotations, edge sorting,
padding, one-hot packing, weight-matrix preprocessing.
"""
import numpy as np
import ml_dtypes

import concourse.bass as bass
import concourse.bacc as bacc
import concourse.tile as tile
from concourse import mybir
from concourse import ap_utils
from concourse.bass import MemorySpace
from concourse.bass_utils import run_bass_kernel_spmd

BF16 = mybir.dt.bfloat16
F32 = mybir.dt.float32
FP8 = mybir.dt.float8e4
I16 = mybir.dt.int16
P = 128
NCORES = 8
ROWB = 256         # table row stride in bf16 elems (512B)
PAYB = 132         # gathered payload in bf16 elems (264B): h(128) + esrc(4)
HALF = 17408       # A-table rows (written early so A-gathers start ~40us in;
                   # B width 50176-17408=32768... must be < 32768 for int16)
GA = 2             # phase-A tiles per group; also windows per edge-pair
NGS = 8            # phase-A groups per slab (one x load + table write each)
AHEAD_A = 10       # A-half gathers issued this many pairs ahead
AHEAD_B = 3        # B-half gathers issued this many pairs ahead


def cfg_full():
    return dict(N=50000, E=800000, IN_CH=128, HID=64, HEADS=4, G=64, OUT2=256)


def _wrap_idx(idx):
    """[n] int -> [128, n//16] int16 gather layout (16-wrap, replicated 8x)."""
    n = idx.shape[0]
    assert n % 16 == 0
    return np.tile(idx.reshape(n // 16, 16).T, (8, 1)).astype(np.int16)


def _pairs(n_win, group=GA):
    return [tuple(range(i, min(i + group, n_win)))
            for i in range(0, n_win, group)]


def raw_dma_gather(eng, out_ap, in_ap, idxs_ap, num_idxs, elem_size, elem_step,
                   queue_num=0):
    """dma_gather without the elem_size%256 restriction (non-transpose)."""
    assert idxs_ap.dtype == mybir.dt.int16
    assert in_ap.dtype == out_ap.dtype
    assert in_ap.space == MemorySpace.DRAM
    assert ap_utils.ap_is_contiguous(in_ap.ap[1:])
    assert ap_utils.ap_is_contiguous(out_ap.ap[1:])
    assert in_ap.ap[-1][1] == out_ap.ap[-1][1] == elem_size
    assert in_ap.ap[0][0] == elem_step
    stride_bytes = elem_step * mybir.dt.size(in_ap.dtype)
    assert stride_bytes % 256 == 0
    _in_ap = eng.lower_ap_dma(in_ap, for_custom_bir_dma=True)
    return eng.add_instruction(
        mybir.InstDMAGatherAnt(
            name=eng.bass.get_next_instruction_name(),
            ins=[*_in_ap, eng.lower_ap(idxs_ap),
                 eng.lower_val_access(eng.to_reg(num_idxs))],
            outs=[eng.lower_ap(out_ap)],
            transpose=False, num_idxs=num_idxs, elem_size=elem_size,
            stride_bytes_256=stride_bytes // 256,
            gen_mode=0, single_packet=False, queue_num=queue_num,
            sbuf_tokens_per_rank=0, sbuf_free_dim_per_rank=0,
            sbuf_free_dim_pad_per_rank=0, sbuf_byte_offset=0,
        ))


class EdgePlan:
    """Host-side per-core edge layout for one graph (shared by both layers)."""

    def __init__(self, src, dst, N, core):
        base = core * (N // NCORES)
        nloc = N // NCORES
        self.n_win = (nloc + P - 1) // P
        m = (dst >= base) & (dst < base + nloc)
        s, d = src[m], dst[m] - base
        order = np.argsort(d, kind="stable")
        s, d = s[order], d[order]
        rot = (s - base) % N          # rotated src row
        self.win_edges = []           # per window: (rotA, rotB, cA, cB)
        for w in range(self.n_win):
            lo, hi = np.searchsorted(d, [w * P, (w + 1) * P])
            rs, dl = rot[lo:hi], d[lo:hi]
            h1 = rs < HALF
            self.win_edges.append((rs[h1], rs[~h1] - HALF,
                                   dl[h1] - w * P, dl[~h1] - w * P))

    @staticmethod
    def tile_counts(plans):
        n_win = plans[0].n_win
        T1, T2 = [], []
        for w in range(n_win):
            m1 = max(len(p.win_edges[w][0]) for p in plans)
            m2 = max(len(p.win_edges[w][1]) for p in plans)
            T1.append(max(1, -(-m1 // P)))
            T2.append(max(1, -(-m2 // P)))
        return T1, T2

    def arrays(self, T1, T2):
        """fatA, fatB idx arrays + packed fp8 one-hots oh_all / ohT_all.

        Per pair, tile order A(w0)|A(w1)|B(w0)|B(w1). oh[e, d*tt+t]=1 iff
        edge (t,e) has local dst d; ohT[d, t*128+e] likewise. Pad edges get
        all-zero one-hot columns (their gathered rows are ignored)."""
        f8 = ml_dtypes.float8_e4m3fn
        fatA, fatB, ohs, ohTs = [], [], [], []
        for pw in _pairs(self.n_win):
            a_idx, b_idx, dls = [], [], [[], []]
            for w in pw:
                rsA, rsB, cA, cB = self.win_edges[w]
                n1, n2 = T1[w] * P, T2[w] * P
                a = np.zeros(n1, np.int64); a[:len(rsA)] = rsA
                b = np.zeros(n2, np.int64); b[:len(rsB)] = rsB
                ca = np.full(n1, -1, np.int64); ca[:len(cA)] = cA
                cb = np.full(n2, -1, np.int64); cb[:len(cB)] = cB
                a_idx.append(a); b_idx.append(b)
                dls[0].append(ca); dls[1].append(cb)
            fatA.append(_wrap_idx(np.concatenate(a_idx)))
            fatB.append(_wrap_idx(np.concatenate(b_idx)))
            dl = np.concatenate(dls[0] + dls[1])      # [tt*128] local dst
            tt = dl.shape[0] // P
            oh = np.zeros((P, P * tt), f8)
            ohT = np.zeros((P, P * tt), f8)
            for t in range(tt):
                dt_ = dl[t * P:(t + 1) * P]
                v = dt_ >= 0
                e = np.nonzero(v)[0]
                oh[e, t * P + dt_[v]] = 1       # tile-major for DoubleRow lhsT
                ohT[dt_[v], t * P + e] = 1
            ohs.append(oh); ohTs.append(ohT)
        return (np.concatenate(fatA, 1), np.concatenate(fatB, 1),
                np.concatenate(ohs, 1), np.concatenate(ohTs, 1))


def _pair_meta(T1, T2, n_win):
    meta = []
    for pw in _pairs(n_win):
        tA = [T1[w] for w in pw]
        tB = [T2[w] for w in pw]
        twin = []
        for i, w in enumerate(pw):
            twin += [i] * tA[i]
        for i, w in enumerate(pw):
            twin += [i] * tB[i]
        meta.append((pw, twin, tA, tB))
    return meta


def build_layer(C, n_nodes_pad, T1, T2, layer, pool_meta=None):
    K = C["IN_CH"] if layer == 1 else C["HEADS"] * C["HID"]
    HO = C["HEADS"] * C["HID"]          # 256
    H = C["HEADS"]; HID = C["HID"]
    nloc = C["N"] // NCORES
    n_win = (nloc + P - 1) // P
    nloc_pad = n_win * P
    ntiles = sum(T1) + sum(T2)
    NROWS = n_nodes_pad
    G = C["G"]
    EXTW = HO + 2 * H                    # 264
    kh = K // P
    pmeta = _pair_meta(T1, T2, n_win)
    ntn = NROWS // P
    NB = NROWS - HALF                    # B-half rows
    gsplit = HALF // (GA * P)            # first phase-A group of the A half

    nc = bacc.Bacc("TRN2", debug=False, num_devices=NCORES,
                   num_swdge_queues=4, dynamic_dma_scratch_size=32768)

    # kh==2 uses fp8 DoubleRow phase A: x halves + x16-scaled weights in fp8
    XDT = BF16 if kh == 1 else FP8
    xT_res = nc.dram_tensor("xT_res", [P, NROWS], XDT, kind="ExternalInput")
    if kh == 2:
        xT_str = nc.dram_tensor("xT_str", [P, NROWS], XDT, kind="ExternalInput")
    rhs_ext_d = nc.dram_tensor("rhs_ext", [P, kh * EXTW], XDT, kind="ExternalInput")
    bias_bc_d = nc.dram_tensor("bias_bc", [P, HO], BF16, kind="ExternalInput")
    fatA_d = nc.dram_tensor("fatA", [P, sum(T1) * 8], I16, kind="ExternalInput")
    fatB_d = nc.dram_tensor("fatB", [P, sum(T2) * 8], I16, kind="ExternalInput")
    oh_d = nc.dram_tensor("oh", [P, P * ntiles], FP8, kind="ExternalInput")
    ohT_d = nc.dram_tensor("ohT", [P, P * ntiles], FP8, kind="ExternalInput")
    tableA = nc.dram_tensor("tableA", [HALF * ROWB], BF16, kind="Internal")
    tableB = nc.dram_tensor("tableB", [NB * ROWB], BF16, kind="Internal")
    h_own = nc.dram_tensor("h_own", [nloc_pad, HO], BF16, kind="ExternalOutput")
    if pool_meta:
        x_own = nc.dram_tensor("x_own", [nloc_pad, C["IN_CH"]], BF16, kind="ExternalInput")
        h1_own = nc.dram_tensor("h1_own", [nloc_pad, HO], BF16, kind="ExternalInput")
        goh_d = nc.dram_tensor("goh", [P, n_win * G], FP8, kind="ExternalInput")
        pool5 = nc.dram_tensor("pool5", [5 * P, G], F32, kind="ExternalOutput")

    viewA = bass.AP(tableA, 0, [[ROWB, HALF], [1, PAYB]])
    viewB = bass.AP(tableB, 0, [[ROWB, NB], [1, PAYB]])

    with tile.TileContext(nc) as tc:
        ctx_pools = []

        def open_pool(**kw):
            cm = tc.tile_pool(**kw)
            pool = cm.__enter__()
            ctx_pools.append(cm)
            return pool

        resP = open_pool(name="res", bufs=1)
        bias_bc = resP.tile([P, HO], BF16)
        nc.sync.dma_start(bias_bc[:], bias_bc_d[:, :])
        rhs_ext = resP.tile([P, kh * EXTW], XDT)
        nc.sync.dma_start(rhs_ext[:], rhs_ext_d[:, :])
        edst_sb = resP.tile([P, n_win * H], BF16)
        idxP = open_pool(name="idx", bufs=1)
        fatA_sb = idxP.tile([P, sum(T1) * 8], I16)
        nc.sync.dma_start(fatA_sb[:], fatA_d[:, :])
        fatB_sb = idxP.tile([P, sum(T2) * 8], I16)
        nc.sync.dma_start(fatB_sb[:], fatB_d[:, :])
        if pool_meta:
            goh_sb = resP.tile([P, n_win * G], FP8)
            nc.sync.dma_start(goh_sb[:], goh_d[:, :])

        eg_ga = open_pool(name="eg_ga", bufs=AHEAD_A + 2)
        eg_gb = open_pool(name="eg_gb", bufs=AHEAD_B + 2)
        npair = len(pmeta)
        colAs, colBs = [0], [0]
        for _, _, tA, tB in pmeta:
            colAs.append(colAs[-1] + sum(tA) * 8)
            colBs.append(colBs[-1] + sum(tB) * 8)

        def emit_gatherA(ip):
            ta = sum(pmeta[ip][2])
            gbufA = eg_ga.tile([P, ta * PAYB], BF16, tag="gbufA")
            raw_dma_gather(
                nc.gpsimd, gbufA[:].rearrange("p (t e) -> p t e", e=PAYB),
                viewA, fatA_sb[:, colAs[ip]:colAs[ip] + ta * 8],
                ta * P, PAYB, ROWB, queue_num=ip % 4)
            return gbufA

        def emit_gatherB(ip):
            tb = sum(pmeta[ip][3])
            gbufB = eg_gb.tile([P, tb * PAYB], BF16, tag="gbufB")
            raw_dma_gather(
                nc.gpsimd, gbufB[:].rearrange("p (t e) -> p t e", e=PAYB),
                viewB, fatB_sb[:, colBs[ip]:colBs[ip] + tb * 8],
                tb * P, PAYB, ROWB, queue_num=(ip + 2) % 4)
            return gbufB

        # ---------- phase A: node transform -> fat tables ----
        # NGS-group slabs: one x load + one table write DMA per slab.
        # Node mapping within a slab: row s0*GA*P + p*ngga + m is produced by
        # partition p of matmul m, so each partition writes ngga contiguous
        # table rows. The A-half slabs are emitted first, then the first
        # AHEAD_A pairs' A-half gathers (deps are per-engine op COUNTERS, so
        # overlap requires emission-order interleaving), then the B slabs.
        gbufAs = {}
        with tc.tile_pool(name="pa_ps", bufs=4, space="PSUM") as pa_ps, \
             tc.tile_pool(name="pa_x", bufs=3) as pa_x, \
             tc.tile_pool(name="pa_e", bufs=3) as pa_e:

            def emit_slab(s0, limit):
                ng = min(NGS, limit - s0)
                cw = ng * GA * P
                ngga = ng * GA
                tbl, roff = (tableA, s0 * GA * P) if s0 < gsplit else \
                            (tableB, (s0 - gsplit) * GA * P)
                if kh == 1:
                    xa = pa_x.tile([P, cw], BF16, tag="xa")
                    nc.sync.dma_start(xa[:],
                                      xT_res[:, s0 * GA * P:s0 * GA * P + cw])
                else:
                    # one tile, k-tile-major: [0,cw)=rows 0-127, [cw,2cw)=128-255
                    xa = pa_x.tile([P, 2 * cw], FP8, tag="xa")
                    nc.sync.dma_start(xa[:, 0:cw],
                                      xT_res[:, s0 * GA * P:s0 * GA * P + cw])
                    nc.sync.dma_start(xa[:, cw:2 * cw],
                                      xT_str[:, s0 * GA * P:s0 * GA * P + cw])
                # full 512B rows (pad included) so the slab write is one
                # contiguous 8KB run per partition instead of 272B packets
                ext = pa_e.tile([P, ngga * ROWB], BF16, tag="ext")
                exs = ext[:]
                ext8 = exs.bitcast(FP8)
                xas = xa[:]
                res_ = rhs_ext[:]
                for gi in range(ng):
                    ps = pa_ps.tile([P, GA * 512], F32, tag="pa")
                    for j in range(GA):
                        xcol = (gi * GA + j) * P
                        if kh == 1:
                            nc.tensor.matmul(ps[:, j * 512:j * 512 + EXTW],
                                             xa[:, xcol:xcol + P],
                                             rhs_ext[:, 0:EXTW],
                                             start=True, stop=True)
                        else:
                            nc.tensor.matmul(
                                ps[:, j * 512:j * 512 + EXTW],
                                bass.AP(xas.tensor, xas.offset + xcol,
                                        [xas.ap[0], [cw, 2], [1, P]]),
                                bass.AP(res_.tensor, res_.offset,
                                        [res_.ap[0], [EXTW, 2], [1, EXTW]]),
                                start=True, stop=True,
                                perf_mode=mybir.MatmulPerfMode.DoubleRow)
                    pss = ps[:]
                    usc = 1.0 if kh == 1 else 1.0 / 16.0
                    # h f32 -> fp8 bytes (unscale the x16 fp8 weights)
                    nc.scalar.activation(
                        bass.AP(ext8.tensor, ext8.offset + gi * GA * 2 * ROWB,
                                [ext8.ap[0], [2 * ROWB, GA], [1, HO]]),
                        bass.AP(pss.tensor, pss.offset,
                                [pss.ap[0], [512, GA], [1, HO]]),
                        mybir.ActivationFunctionType.Identity, scale=usc)
                    # esrc+edst f32 -> bf16 at cols [128,136)
                    nc.scalar.activation(
                        bass.AP(exs.tensor, exs.offset + gi * GA * ROWB + P,
                                [exs.ap[0], [ROWB, GA], [1, 8]]),
                        bass.AP(pss.tensor, pss.offset + HO,
                                [pss.ap[0], [512, GA], [1, 8]]),
                        mybir.ActivationFunctionType.Identity, scale=usc)
                nc.scalar.dma_start(
                    bass.AP(tbl, roff * ROWB,
                            [[ngga * ROWB, P], [1, ngga * ROWB]]), ext[:])

            for s0 in range(0, gsplit, NGS):
                emit_slab(s0, gsplit)
            # tableA complete: local-edst readback + early A-half gathers go
            # to gpsimd now so desc-gen/DMA overlap the B-half of phase A
            nc.gpsimd.dma_start(
                edst_sb[:],
                bass.AP(tableA, PAYB, [[ROWB, P], [P * ROWB, n_win], [1, H]]))
            for ip in range(min(AHEAD_A, npair)):
                gbufAs[ip] = emit_gatherA(ip)
            for s0 in range(gsplit, ntn // GA, NGS):
                emit_slab(s0, ntn // GA)

        # ---------- edge phase ----------
        with tc.tile_pool(name="eg_ps", bufs=3, space="PSUM") as eg_ps, \
             tc.tile_pool(name="ed_ps", bufs=3, space="PSUM") as ed_ps, \
             (tc.tile_pool(name="pool_ps", bufs=2, space="PSUM") if pool_meta
              else tc.tile_pool(name="dummy", bufs=1)) as pl_ps, \
             tc.tile_pool(name="eg_g2", bufs=2) as eg_g2, \
             tc.tile_pool(name="eg_o", bufs=3) as eg_o, \
             tc.tile_pool(name="eg_s", bufs=3) as eg_s, \
             tc.tile_pool(name="pl_in", bufs=3) as pl_in:
            if pool_meta:
                pool_acc = resP.tile([P, 5 * G], F32)
                nc.vector.memset(pool_acc[:], 0.0)

            def emit_front(ip, tile0, gbufA, gbufB):
                pw, twin, tA, tB = pmeta[ip]
                nw = len(pw)
                ta, tb = sum(tA), sum(tB)
                tt = ta + tb
                oh_sb = eg_o.tile([P, P * tt], FP8, tag="ohsb")
                nc.sync.dma_start(oh_sb[:], oh_d[:, P * tile0:P * (tile0 + tt)])
                ohT_sb = eg_o.tile([P, P * tt], FP8, tag="ohTsb")
                nc.sync.dma_start(ohT_sb[:], ohT_d[:, P * tile0:P * (tile0 + tt)])
                xo = h1o = None
                if pool_meta:
                    w0 = pw[0]
                    IN = C["IN_CH"]
                    xo = pl_in.tile([P, nw * IN], BF16, tag="xo")
                    nc.sync.dma_start(
                        xo[:], bass.AP(x_own, w0 * IN,
                                       [[n_win * IN, P], [1, nw * IN]]))
                    h1o = pl_in.tile([P, nw * HO], BF16, tag="h1o")
                    nc.sync.dma_start(
                        h1o[:], bass.AP(h1_own, w0 * HO,
                                        [[n_win * HO, P], [1, nw * HO]]))

                # per-edge dst logits via ohT matmuls, two tiles per pass:
                # DoubleRow with rhs [edst_wa|0 ; 0|edst_wb] -> [eps_t|eps_t1]
                edp = eg_s.tile([P, 4 * 4 * H], FP8, tag="edp")
                nc.vector.memset(edp[:], 0.0)
                combos = {}
                for q in range(tt // 2):
                    wa, wb = pw[twin[2 * q]], pw[twin[2 * q + 1]]
                    if (wa, wb) not in combos:
                        v = len(combos)
                        combos[(wa, wb)] = v
                        nc.scalar.copy(edp[:, v * 16:v * 16 + H],
                                       edst_sb[:, wa * H:(wa + 1) * H])
                        nc.scalar.copy(edp[:, v * 16 + 12:v * 16 + 16],
                                       edst_sb[:, wb * H:(wb + 1) * H])
                edps = edp[:]
                ohTs = ohT_sb[:]
                eps = ed_ps.tile([P, tt * H], F32, tag="eps")
                for q in range(tt // 2):
                    v = combos[(pw[twin[2 * q]], pw[twin[2 * q + 1]])]
                    nc.tensor.matmul(
                        eps[:, q * 8:q * 8 + 8],
                        bass.AP(ohTs.tensor, ohTs.offset + 2 * q * P,
                                [ohTs.ap[0], [P, 2], [1, P]]),
                        bass.AP(edps.tensor, edps.offset + v * 16,
                                [edps.ap[0], [8, 2], [1, 8]]),
                        start=True, stop=True,
                        perf_mode=mybir.MatmulPerfMode.DoubleRow)
                if tt % 2:
                    t = tt - 1
                    w = pw[twin[t]]
                    nc.tensor.matmul(eps[:, t * H:(t + 1) * H],
                                     ohT_sb[:, t * P:(t + 1) * P],
                                     edst_sb[:, w * H:(w + 1) * H],
                                     start=True, stop=True)

                # logit = esrc + eps, lrelu, exp -> exb [P, tt*H] bf16
                logit = eg_s.tile([P, tt * H], F32, tag="logit")
                for gb, t0, tn in ((gbufA, 0, ta), (gbufB, ta, tb)):
                    gbs = gb[:]
                    esrc_ap = bass.AP(gbs.tensor, gbs.offset + P,
                                      [gbs.ap[0], [PAYB, tn], [1, H]])
                    nc.vector.tensor_tensor(
                        out=logit[:, t0 * H:(t0 + tn) * H], in0=esrc_ap,
                        in1=eps[:, t0 * H:(t0 + tn) * H], op=mybir.AluOpType.add)
                lr = eg_s.tile([P, tt * H], F32, tag="lrelu")
                nc.vector.scalar_tensor_tensor(
                    out=lr[:], in0=logit[:], scalar=0.2, in1=logit[:],
                    op0=mybir.AluOpType.mult, op1=mybir.AluOpType.max)
                exb = eg_s.tile([P, tt * H], BF16, tag="exb")
                nc.scalar.activation(exb[:], lr[:],
                                     mybir.ActivationFunctionType.Exp)
                # gbuf2 = [h*ex (256) | ex (4)] per tile, fp8 (DoubleRow rhs)
                gbuf2 = eg_g2.tile([P, tt * 260], FP8, tag="gbuf2")
                g2s = gbuf2[:]
                exs = exb[:]
                for gb, t0, tn in ((gbufA, 0, ta), (gbufB, ta, tb)):
                    g8 = gb[:].bitcast(FP8)
                    nc.vector.tensor_tensor(
                        out=bass.AP(g2s.tensor, g2s.offset + t0 * 260,
                                    [g2s.ap[0], [260, tn], [HID, H], [1, HID]]),
                        in0=bass.AP(g8.tensor, g8.offset,
                                    [g8.ap[0], [2 * PAYB, tn], [HID, H], [1, HID]]),
                        in1=bass.AP(exs.tensor, exs.offset + t0 * H,
                                    [exs.ap[0], [H, tn], [1, H], [0, HID]]),
                        op=mybir.AluOpType.mult)
                nc.scalar.copy(
                    bass.AP(g2s.tensor, g2s.offset + HO,
                            [g2s.ap[0], [260, tt], [1, H]]), exb[:])
                return (pw, twin, tt, oh_sb, gbuf2, xo, h1o)

            def emit_back(state):
                pw, twin, tt, oh_sb, gbuf2, xo, h1o = state
                nw = len(pw)
                hb = eg_s.tile([P, nw * HO], BF16, tag="hb")
                ohs = oh_sb[:]
                g2s = gbuf2[:]
                for i, w in enumerate(pw):
                    ps = eg_ps.tile([P, HO + H], F32, tag="egps")
                    tiles = [t for t in range(tt) if twin[t] == i]
                    # pair CONSECUTIVE tiles into fp8 DoubleRow matmuls
                    # (0.5 cycles/out-col, 2 edge-tiles contracted per pass)
                    runs = []
                    for t in tiles:
                        if runs and runs[-1][-1] == t - 1:
                            runs[-1].append(t)
                        else:
                            runs.append([t])
                    mms = []
                    for run in runs:
                        q = 0
                        while q + 1 < len(run):
                            mms.append((run[q], 2))
                            q += 2
                        if q < len(run):
                            mms.append((run[q], 1))
                    for k, (t, span) in enumerate(mms):
                        st = (k == 0)
                        sp = (k == len(mms) - 1)
                        if span == 2:
                            nc.tensor.matmul(
                                ps[:],
                                bass.AP(ohs.tensor, ohs.offset + t * P,
                                        [ohs.ap[0], [P, 2], [1, P]]),
                                bass.AP(g2s.tensor, g2s.offset + t * 260,
                                        [g2s.ap[0], [260, 2], [1, 260]]),
                                start=st, stop=sp,
                                perf_mode=mybir.MatmulPerfMode.DoubleRow)
                        else:
                            nc.tensor.matmul(
                                ps[:], oh_sb[:, t * P:(t + 1) * P],
                                gbuf2[:, t * 260:t * 260 + 260],
                                start=st, stop=sp)
                    den = eg_s.tile([P, H], F32, tag="den")
                    nc.vector.tensor_scalar(out=den[:], in0=ps[:, HO:HO + H],
                                            scalar1=1e-30, scalar2=None,
                                            op0=mybir.AluOpType.add)
                    rec = eg_s.tile([P, H], F32, tag="rec")
                    nc.vector.reciprocal_approx_fast(out=rec[:], in_=den[:])
                    rcs = rec[:]
                    nc.vector.tensor_tensor(
                        out=hb[:, i * HO:(i + 1) * HO], in0=ps[:, 0:HO],
                        in1=bass.AP(rcs.tensor, rcs.offset,
                                    [rcs.ap[0], [1, H], [0, HID]]),
                        op=mybir.AluOpType.mult)
                hbs = hb[:]
                bbs = bias_bc[:]
                nc.vector.tensor_tensor(
                    out=hbs, in0=hbs,
                    in1=bass.AP(bbs.tensor, bbs.offset,
                                [bbs.ap[0], [0, nw], [1, HO]]),
                    op=mybir.AluOpType.add)
                nc.scalar.activation(hbs, hbs,
                                     mybir.ActivationFunctionType.Relu)
                w0 = pw[0]
                nc.scalar.dma_start(
                    bass.AP(h_own, w0 * HO, [[n_win * HO, P], [1, nw * HO]]),
                    hbs)

                if pool_meta:
                    for i, w in enumerate(pw):
                        psc = pl_ps.tile([P, 5 * G], F32, tag="psc")
                        nc.tensor.matmul(psc[:, 0:G],
                                         xo[:, i * C["IN_CH"]:(i + 1) * C["IN_CH"]],
                                         goh_sb[:, w * G:(w + 1) * G],
                                         start=True, stop=True)
                        for j in range(2):
                            nc.tensor.matmul(
                                psc[:, (1 + j) * G:(2 + j) * G],
                                h1o[:, i * HO + j * P:i * HO + (j + 1) * P],
                                goh_sb[:, w * G:(w + 1) * G], start=True, stop=True)
                            nc.tensor.matmul(
                                psc[:, (3 + j) * G:(4 + j) * G],
                                hb[:, i * HO + j * P:i * HO + (j + 1) * P],
                                goh_sb[:, w * G:(w + 1) * G], start=True, stop=True)
                        nc.vector.tensor_tensor(out=pool_acc[:], in0=pool_acc[:],
                                                in1=psc[:], op=mybir.AluOpType.add)

            # pipeline: gathers pre-issued AHEAD_{A,B} pairs; front(k) before
            # back(k-1). The first AHEAD_A A-gathers were emitted mid-phase-A.
            gbufBs = {}
            for ip in range(min(AHEAD_B, npair)):
                gbufBs[ip] = emit_gatherB(ip)
            tile0 = 0
            prev = None
            for ip, (pw, twin, tA, tB) in enumerate(pmeta):
                if ip + AHEAD_A < npair:
                    gbufAs[ip + AHEAD_A] = emit_gatherA(ip + AHEAD_A)
                if ip + AHEAD_B < npair:
                    gbufBs[ip + AHEAD_B] = emit_gatherB(ip + AHEAD_B)
                state = emit_front(ip, tile0, gbufAs.pop(ip), gbufBs.pop(ip))
                if prev is not None:
                    emit_back(prev)
                prev = state
                tile0 += sum(tA) + sum(tB)
            if prev is not None:
                emit_back(prev)
            if pool_meta:
                nc.sync.dma_start(
                    bass.AP(pool5, 0, [[G, P], [P * G, 5], [1, G]]), pool_acc[:])
        for p in reversed(ctx_pools):
            p.__exit__(None, None, None)
    nc.compile()
    return nc


def build_l3(C):
    """MLP head on 1 core, single packed input DMA. Output transposed [128, G]."""
    G = C["G"]; DC = C["IN_CH"] + 2 * C["HEADS"] * C["HID"]  # 640
    K5 = DC // P                                             # 5
    # packed cols: pa (8*K5*G) | w3 (K5*256) | w4 (256) | recbc (G) | b3 (2) | b4 (1)
    CPA = NCORES * K5 * G
    CW3 = K5 * 256
    CTOT = CPA + CW3 + 256 + G + 2 + 1
    nc = bacc.Bacc("TRN2", debug=False, num_devices=1)
    packed = nc.dram_tensor("packed", [P, CTOT], F32, kind="ExternalInput")
    out = nc.dram_tensor("out", [P, G], F32, kind="ExternalOutput")

    with tile.TileContext(nc) as tc:
        with tc.tile_pool(name="sb", bufs=1) as sb, \
             tc.tile_pool(name="ps", bufs=1, space="PSUM") as psp:
            pk = sb.tile([P, CTOT], F32)
            nc.sync.dma_start(pk[:], packed[:, :])
            OW3 = CPA
            OW4 = CPA + CW3
            OREC = OW4 + 256
            OB3 = OREC + G
            OB4 = OB3 + 2

            half = CPA // 2
            nc.vector.tensor_tensor(out=pk[:, 0:half], in0=pk[:, 0:half],
                                    in1=pk[:, half:2 * half], op=mybir.AluOpType.add)
            nc.vector.tensor_tensor(out=pk[:, 0:half // 2], in0=pk[:, 0:half // 2],
                                    in1=pk[:, half // 2:half], op=mybir.AluOpType.add)
            nc.vector.tensor_tensor(out=pk[:, 0:half // 4], in0=pk[:, 0:half // 4],
                                    in1=pk[:, half // 4:half // 2],
                                    op=mybir.AluOpType.add)
            pooled = sb.tile([P, K5 * G], F32)
            rcs = pk[:, OREC:OREC + G]
            nc.vector.tensor_tensor(
                out=pooled[:], in0=pk[:, 0:K5 * G],
                in1=bass.AP(rcs.tensor, rcs.offset, [rcs.ap[0], [0, K5], [1, G]]),
                op=mybir.AluOpType.mult)

            hm = sb.tile([P, 2 * G], F32)
            for j in range(2):
                ps1 = psp.tile([P, G], F32, tag=f"mm{j}")
                for i in range(K5):
                    nc.tensor.matmul(
                        ps1[:],
                        pk[:, OW3 + i * 256 + j * P:OW3 + i * 256 + (j + 1) * P],
                        pooled[:, i * G:(i + 1) * G],
                        start=(i == 0), stop=(i == K5 - 1))
                nc.scalar.activation(hm[:, j * G:(j + 1) * G], ps1[:],
                                     mybir.ActivationFunctionType.Relu,
                                     bias=pk[:, OB3 + j:OB3 + j + 1], scale=1.0)
            ps2 = psp.tile([P, G], F32, tag="mm2")
            for j in range(2):
                nc.tensor.matmul(ps2[:], pk[:, OW4 + j * P:OW4 + (j + 1) * P],
                                 hm[:, j * G:(j + 1) * G],
                                 start=(j == 0), stop=(j == 1))
            ot = sb.tile([P, G], F32)
            nc.scalar.activation(ot[:], ps2[:],
                                 mybir.ActivationFunctionType.Identity,
                                 bias=pk[:, OB4:OB4 + 1], scale=1.0)
            nc.sync.dma_start(out[:, :], ot[:])
    nc.compile()
    return nc


def _prep_rot_xT(x, base, n_pad):
    """x [N, K] f32 -> rotated, phase-A slab-interleaved xT [K, n_pad] bf16.

    Slab-local column m*P + p holds node pos + p*ngga + m, so phase-A matmul m
    yields psum partition p = node p*ngga+m and each partition's slab rows are
    contiguous in the table."""
    N, K = x.shape
    xr = np.zeros((n_pad, K), np.float32)
    idx = (base + np.arange(N)) % N
    xr[:N] = x[idx]
    xT = np.ascontiguousarray(xr.T)
    SL = NGS * GA * P
    out = np.empty_like(xT)
    # slab boundaries mirror the device: A slabs clip at HALF, B slabs after
    bounds = list(range(0, HALF, SL)) + list(range(HALF, n_pad, SL)) + [n_pad]
    for pos, end in zip(bounds[:-1], bounds[1:]):
        cw = end - pos
        ngga = cw // P
        out[:, pos:pos + cw] = (xT[:, pos:pos + cw]
                                .reshape(K, P, ngga).transpose(0, 2, 1)
                                .reshape(K, cw))
    return np.ascontiguousarray(out)


def _flat_att(a):
    """[H, D] attention vec -> [H*D, H] block matrix."""
    H, D = a.shape
    A = np.zeros((H * D, H), np.float32)
    for h in range(H):
        A[h * D:(h + 1) * D, h] = a[h]
    return A


def _rhs_ext(W, a_src, a_dst, scale=1.0):
    """[K//P*P rows..., 264*kh]: k-half j = [W_j | (W@A_src)_j | (W@A_dst)_j]."""
    W = np.asarray(W, np.float32)
    K, HO = W.shape
    As = W @ _flat_att(np.asarray(a_src, np.float32))
    Ad = W @ _flat_att(np.asarray(a_dst, np.float32))
    full = np.concatenate([W, As, Ad], 1) * scale       # [K, 264]
    kh = K // P
    return np.concatenate([full[j * P:(j + 1) * P] for j in range(kh)], 1)


def kernel(x, edge_index, batch, W1, a1_src, a1_dst, b1, W2, a2_src, a2_dst, b2,
           W3, b3, W4, b4, _trace=False, _timings=None):
    C = cfg_full()
    N, E, G = C["N"], C["E"], C["G"]
    IN, HO = C["IN_CH"], C["HEADS"] * C["HID"]
    nloc = N // NCORES
    n_win = (nloc + P - 1) // P
    nloc_pad = n_win * P

    x = np.asarray(x, np.float32)
    src = np.asarray(edge_index[0], np.int64)
    dst = np.asarray(edge_index[1], np.int64)
    batch = np.asarray(batch, np.int64)
    bf = ml_dtypes.bfloat16
    f8 = ml_dtypes.float8_e4m3fn

    ntn = -(-N // P) + 1
    NROWS = ntn * P

    plans = [EdgePlan(src, dst, N, c) for c in range(NCORES)]
    T1, T2 = EdgePlan.tile_counts(plans)
    earr = [p.arrays(T1, T2) for p in plans]

    bias1 = np.tile(np.asarray(b1, np.float32).reshape(1, HO), (P, 1)).astype(bf)
    bias2 = np.tile(np.asarray(b2, np.float32).reshape(1, HO), (P, 1)).astype(bf)
    rext1 = _rhs_ext(W1, a1_src, a1_dst).astype(bf)
    rext2 = _rhs_ext(W2, a2_src, a2_dst, scale=16.0).astype(f8)

    # ---------------- L1 ----------------
    nc1 = build_layer(C, NROWS, T1, T2, layer=1)
    in_maps = []
    for c in range(NCORES):
        fA, fB, oh, ohT = earr[c]
        in_maps.append({
            "xT_res": _prep_rot_xT(x, c * nloc, NROWS).astype(bf),
            "rhs_ext": rext1, "bias_bc": bias1,
            "fatA": fA, "fatB": fB, "oh": oh, "ohT": ohT,
        })
    r1 = run_bass_kernel_spmd(nc1, in_maps, core_ids=list(range(NCORES)), trace=_trace)
    if _timings is not None and r1.exec_time_ns:
        _timings.append(("L1", r1.exec_time_ns))

    def _unpack_own(h):
        # device layout [p][w][c] -> [node, c]
        return (h.astype(np.float32).reshape(P, n_win, HO)
                .transpose(1, 0, 2).reshape(nloc_pad, HO)[:nloc])

    h1_full = np.concatenate(
        [_unpack_own(r1.results[c]["h_own"]) for c in range(NCORES)])

    # ---------------- L2 ----------------
    nc2 = build_layer(C, NROWS, T1, T2, layer=2, pool_meta=True)
    in_maps2 = []
    for c in range(NCORES):
        fA, fB, oh, ohT = earr[c]
        base = c * nloc
        h1T = _prep_rot_xT(h1_full, base, NROWS)

        def _pack_own(arr, ch):
            # [node, c] (zero-padded) -> device layout [p][w][c]
            full = np.zeros((nloc_pad, ch), np.float32)
            full[:nloc] = arr
            return (full.reshape(n_win, P, ch).transpose(1, 0, 2)
                    .reshape(nloc_pad, ch))

        xo = _pack_own(x[base:base + nloc], IN)
        h1o = _pack_own(h1_full[base:base + nloc], HO)
        goh = np.zeros((P, n_win * G), f8)
        gi = batch[base:base + nloc]
        rows = np.arange(nloc) % P
        wins = np.arange(nloc) // P
        goh[rows, wins * G + gi] = 1
        in_maps2.append({
            "xT_res": np.ascontiguousarray(h1T[0:P]).astype(f8),
            "xT_str": np.ascontiguousarray(h1T[P:2 * P]).astype(f8),
            "rhs_ext": rext2, "bias_bc": bias2,
            "fatA": fA, "fatB": fB, "oh": oh, "ohT": ohT,
            "x_own": xo.astype(bf), "h1_own": h1o.astype(bf),
            "goh": goh,
        })
    r2 = run_bass_kernel_spmd(nc2, in_maps2, core_ids=list(range(NCORES)), trace=_trace)
    if _timings is not None and r2.exec_time_ns:
        _timings.append(("L2", r2.exec_time_ns))

    # ---------------- L3 ----------------
    DC = IN + 2 * HO
    K5 = DC // P
    parts = np.concatenate([r2.results[c]["pool5"] for c in range(NCORES)], 0)
    cnt = np.bincount(batch, minlength=G).astype(np.float32)
    rec = (1.0 / np.maximum(cnt, 1.0))
    # pack [P, CTOT]: pa | w3 | w4 | recbc | b3 | b4
    pa = parts.reshape(NCORES * K5, P, G).transpose(1, 0, 2).reshape(P, -1)
    W3f = np.asarray(W3, np.float32)
    w3 = W3f.reshape(K5, P, 256).transpose(1, 0, 2).reshape(P, -1)
    W4f = np.asarray(W4, np.float32)
    w4 = W4f.reshape(2, P, P).transpose(1, 0, 2).reshape(P, -1)
    rec_bc = np.tile(rec.reshape(1, G), (P, 1))
    b3p = np.asarray(b3, np.float32).reshape(2, P).T
    b4p = np.asarray(b4, np.float32).reshape(P, 1)
    packed = np.concatenate([pa, w3, w4, rec_bc, b3p, b4p], 1).astype(np.float32)
    nc3 = build_l3(C)
    r3 = run_bass_kernel_spmd(nc3, [{"packed": packed}], core_ids=[0], trace=_trace)
    if _timings is not None and r3.exec_time_ns:
        _timings.append(("L3", r3.exec_time_ns))
    return np.ascontiguousarray(r3.results[0]["out"].T.astype(np.float32))
